# revision 18
# baseline (speedup 1.0000x reference)
# BitLinear (eval path) Trainium2 kernel: ternary weight quant + int8-grade
# activation quant + dense matmul, tensor-parallel over 8 NeuronCores.
#
# Math (per reference):
#   w_scale[o] = max(mean_k |W[o,k]|, EPS)
#   w_quant    = clip(round(W / w_scale), -1, 1)            (ternary, exact)
#   x_scale[t] = max(max_k |x[t,k]| / 127, EPS)
#   xq[t,k]    = bf16(x[t,k] / x_scale[t])   -- bf16 grid is finer than the
#                int8 grid below 128; skipping the integer round costs ~9e-3
#                rel err vs the int8 reference (budget 2e-2).
#   outT[o,t]  = (sum_k wq[o,k] * xq[t,k]) * ws[o] * xs[t] + bias[o]
#   (outputs computed/stored transposed [O,T] in bf16; host transposes+upcasts)
#
# W ternary stays exact: magic-constant rounding (v + 1.5*2^23), scalar act1
# on ScalarE, act2 (-M -> bf16) on GpSimd, ternary clip fused into the
# post-transpose fp8 cast on DVE.
#
# KEY STRUCTURE (learned from traces of 5 prior revisions):
# - The matmul runs W-STATIONARY / X-MOVING producing outT[o,t]. Each W block
#   (128 rows) then feeds 128 matmuls (~27us of PE work), so the W quant
#   pipeline (1 block per ~11-20us) always stays ahead of the PE -- with the
#   old X-stationary layout each W block fed only 6.8us and the frontend
#   paced the whole kernel (~780us for a 437us-roofline matmul).
# - ws becomes a per-partition scalar in the epilogue (no broadcast needed);
#   xs needs one [1,128]->[128,128] ones-matmul broadcast per token group.
# - All of XQ is resident (128KB/partition bf16, K-major); W streams.
# - Queue split: SWDGE(gpsimd)=x loads; scalar HWDGE=W loads+stores;
#   sync HWDGE=all xbar transposes; frontend at half-K granularity.
import numpy as np

import concourse.bacc as bacc
import concourse.bass as bass
import concourse.tile as tile
from concourse import mybir
from concourse.bass_utils import run_bass_kernel_spmd
from concourse.masks import make_identity

F32 = mybir.dt.float32
BF16 = mybir.dt.bfloat16
FP8 = mybir.dt.float8e4

EPS = 1e-5
MAGIC = 12582912.0  # 1.5 * 2^23

B, S, I, O = 4, 2048, 4096, 4096
T_FULL = B * S
TSPLIT, OSPLIT = 4, 2
N_CORES = TSPLIT * OSPLIT

A = mybir.AluOpType
AF = mybir.ActivationFunctionType


def build_nc(K=I, TO=O // OSPLIT, TT=T_FULL // TSPLIT):
    """Per-core program: x [TT, K], w [TO, K], bias [TO] -> outT [TO, TT]."""
    KT = K // 128      # 32 k subtiles
    KH = K // 2        # 2048: half-K frontend granularity
    KTH = KT // 2      # 16 k subtiles per half
    GT = 128           # tokens / out-rows per group
    NG = TT // GT      # 16 token groups
    NB = TO // GT      # 16 W blocks
    TC = 512           # moving width per matmul (tokens)
    NTC = TT // TC     # 4 token chunks

    nc = bacc.Bacc("TRN2", target_bir_lowering=False, debug=False)
    x_d = nc.dram_tensor("x", [TT, K], F32, kind="ExternalInput").ap()
    w_d = nc.dram_tensor("w", [TO, K], F32, kind="ExternalInput").ap()
    bias_d = nc.dram_tensor("bias", [TO], F32, kind="ExternalInput").ap()
    out_d = nc.dram_tensor("out", [TO, TT], BF16, kind="ExternalOutput").ap()

    with tile.TileContext(nc) as tc:
        with (
            tc.tile_pool(name="lx", bufs=2) as p_lx,      # f32 x half-row loads
            tc.tile_pool(name="lw", bufs=2) as p_lw,      # f32 W half-row loads
            tc.tile_pool(name="btx", bufs=2) as p_btx,    # bf16 quantized x halves
            tc.tile_pool(name="btw", bufs=1) as p_btw,    # bf16 quantized W halves
            tc.tile_pool(name="wst", bufs=1) as p_wst,    # transposed W staging
            tc.tile_pool(name="wq", bufs=5) as p_wq,      # streamed fp8 W blocks
            tc.tile_pool(name="xq", bufs=1) as p_xq,      # resident K-major xq
            tc.tile_pool(name="sml", bufs=6) as p_sml,
            tc.tile_pool(name="osb", bufs=3) as p_osb,
            tc.tile_pool(name="const", bufs=1) as p_const,
            tc.tile_pool(name="ps_mm", bufs=5, space="PSUM") as ps_mm,
            tc.tile_pool(name="ps_tr", bufs=1, space="PSUM") as ps_tr,
        ):
            ident = p_const.tile([128, 128], F32)
            make_identity(nc, ident[:])
            ones_row = p_const.tile([1, 128], BF16)
            nc.vector.memset(ones_row[:], 1.0)
            mag_col = p_const.tile([128, 1], F32)
            nc.vector.memset(mag_col[:], MAGIC)
            xs_cols = p_const.tile([128, NG], F32)    # x_scale, t on partitions
            ws_cols = p_const.tile([128, NB], F32)    # w_scale, o on partitions
            xs_bc = p_const.tile([128, TT], BF16)     # x_scale bcast rows
            bias_cols = p_const.tile([128, NB], F32)  # bias, o on partitions
            nc.sync.dma_start(
                out=bias_cols[:],
                in_=bass.AP(
                    tensor=bias_d.tensor, offset=bias_d.offset,
                    ap=[[1, 128], [128, NB]],
                ),
            )

            xq_ch = [
                p_xq.tile([128, KT, TC], BF16, name=f"xq_{c}") for c in range(NTC)
            ]
            wq_blocks = {}
            lx_tiles = {}
            lw_tiles = {}

            # ---------- loads: half-K tiles, each flow on its own queue ------
            def x_load(tg):
                for h in range(2):
                    xg = p_lx.tile([128, KH], F32, tag="lx")
                    nc.gpsimd.dma_start(
                        out=xg[:],
                        in_=x_d[tg * GT : (tg + 1) * GT, h * KH : (h + 1) * KH],
                    )
                    lx_tiles[(tg, h)] = xg

            def w_load(ob):
                for h in range(2):
                    wg = p_lw.tile([128, KH], F32, tag="lw")
                    nc.scalar.dma_start(
                        out=wg[:],
                        in_=w_d[ob * GT : (ob + 1) * GT, h * KH : (h + 1) * KH],
                    )
                    lw_tiles[(ob, h)] = wg

            # ---------- x group: amax, one-pass quantize, transpose, bcast --
            def x_quant(tg):
                xga = lx_tiles.pop((tg, 0))
                xgb = lx_tiles.pop((tg, 1))
                ama = p_sml.tile([128, 1], F32, tag="am")
                amb = p_sml.tile([128, 1], F32, tag="am")
                nc.vector.tensor_reduce(
                    out=ama[:], in_=xga[:], axis=mybir.AxisListType.X,
                    op=A.max, apply_absolute_value=True,
                )
                nc.vector.tensor_reduce(
                    out=amb[:], in_=xgb[:], axis=mybir.AxisListType.X,
                    op=A.max, apply_absolute_value=True,
                )
                am = p_sml.tile([128, 1], F32, tag="am")
                nc.vector.tensor_tensor(out=am[:], in0=ama[:], in1=amb[:], op=A.max)
                nc.vector.tensor_scalar(
                    out=xs_cols[:, tg : tg + 1], in0=am[:],
                    scalar1=1.0 / 127.0, scalar2=EPS, op0=A.mult, op1=A.max,
                )
                rxs = p_sml.tile([128, 1], F32, tag="rxs")
                nc.vector.reciprocal(rxs[:], xs_cols[:, tg : tg + 1])
                # xs column -> [1,128] row -> K=1 ones-matmul broadcast -> xs_bc
                ptr = ps_tr.tile([1, 128], F32, tag="tr")
                nc.tensor.transpose(ptr[:], xs_cols[:, tg : tg + 1], ident[:])
                xsrow = p_sml.tile([1, 128], BF16, tag="xsrow")
                nc.vector.tensor_copy(xsrow[:], ptr[:])
                pbc = ps_tr.tile([128, 128], F32, tag="bc")
                nc.tensor.matmul(pbc[:], ones_row[:], xsrow[:], start=True, stop=True)
                nc.vector.tensor_copy(xs_bc[:, tg * GT : (tg + 1) * GT], pbc[:])
                # xq = bf16(x * (1/xs)) in ONE scalar pass, then xbar transpose
                ch, tsl = tg // (TC // GT), (tg % (TC // GT)) * GT
                for h, xg in ((0, xga), (1, xgb)):
                    xot = p_btx.tile([128, KH], BF16, tag="btx")
                    nc.scalar.activation(
                        out=xot[:], in_=xg[:], func=AF.Identity, scale=rxs[:],
                    )
                    nc.sync.dma_start_transpose(
                        xq_ch[ch][:, h * KTH : (h + 1) * KTH, tsl : tsl + GT],
                        xot[:],
                    )

            # ---------- W block: scales, ternary, transpose, clip+fp8 -------
            def w_quant(ob):
                wga = lw_tiles.pop((ob, 0))
                wgb = lw_tiles.pop((ob, 1))
                wsa = p_sml.tile([128, 1], F32, tag="ws")
                wsb = p_sml.tile([128, 1], F32, tag="ws")
                nc.vector.tensor_reduce(
                    out=wsa[:], in_=wga[:], axis=mybir.AxisListType.X,
                    op=A.add, apply_absolute_value=True,
                )
                nc.vector.tensor_reduce(
                    out=wsb[:], in_=wgb[:], axis=mybir.AxisListType.X,
                    op=A.add, apply_absolute_value=True,
                )
                wsum = p_sml.tile([128, 1], F32, tag="ws")
                nc.vector.tensor_tensor(out=wsum[:], in0=wsa[:], in1=wsb[:], op=A.add)
                nc.vector.tensor_scalar(
                    out=ws_cols[:, ob : ob + 1], in0=wsum[:],
                    scalar1=1.0 / K, scalar2=EPS, op0=A.mult, op1=A.max,
                )
                rws = p_sml.tile([128, 1], F32, tag="ws")
                nc.vector.reciprocal(rws[:], ws_cols[:, ob : ob + 1])
                # u = w*(1/ws) + M (exact rint, ScalarE); u - M -> bf16 on
                # GpSimd; ternary clip fused into the fp8 cast on DVE
                wqb = p_wq.tile([128, KT, GT], FP8, tag="wq")
                for h, wg in ((0, wga), (1, wgb)):
                    nc.scalar.activation(
                        out=wg[:], in_=wg[:], func=AF.Identity,
                        scale=rws[:], bias=mag_col[:],
                    )
                    wot = p_btw.tile([128, KH], BF16, tag="btw")
                    nc.gpsimd.tensor_scalar(
                        out=wot[:], in0=wg[:], scalar1=-MAGIC, scalar2=0.0,
                        op0=A.add, op1=A.add,
                    )
                    wstg = p_wst.tile([128, KTH, GT], BF16, tag="wst")
                    nc.sync.dma_start_transpose(wstg[:], wot[:])
                    nc.vector.tensor_scalar(
                        out=wqb[:, h * KTH : (h + 1) * KTH, :], in0=wstg[:],
                        scalar1=1.0, scalar2=-1.0, op0=A.min, op1=A.max,
                    )
                wq_blocks[ob] = wqb

            # ---------- matmul unit: one (W block, token chunk) ----------
            def mm_unit(ob, ch):
                wqb = wq_blocks[ob]
                pm = ps_mm.tile([128, TC], F32, tag="mm")
                for kt in range(KT):
                    nc.tensor.matmul(
                        pm[:],
                        wqb[:, kt, :],
                        xq_ch[ch][:, kt, :],
                        start=(kt == 0),
                        stop=(kt == KT - 1),
                    )
                osb = p_osb.tile([128, TC], BF16, tag="osb")
                nc.vector.scalar_tensor_tensor(
                    out=osb[:], in0=pm[:], scalar=ws_cols[:, ob : ob + 1],
                    in1=xs_bc[:, ch * TC : (ch + 1) * TC], op0=A.mult, op1=A.mult,
                )
                nc.scalar.activation(
                    out=osb[:], in_=osb[:], func=AF.Identity,
                    bias=bias_cols[:, ob : ob + 1],
                )
                nc.scalar.dma_start(
                    out=out_d[ob * GT : (ob + 1) * GT, ch * TC : (ch + 1) * TC],
                    in_=osb[:],
                )

            # ---------- main schedule ----------
            # Phase 1: W blocks 0-4 revisited ch-major — the PE sweeps the
            # active 5-block set once per x chunk, arriving at chunk ch just
            # after its x groups land. Block liveness stays within wq bufs=5.
            # Phase 2: blocks 5-15 block-major — each W block covers 27us of
            # PE work vs ~12-17us production, so W stays ahead.
            # X groups are front-loaded hard: they gate the chunk sweeps.
            x_load(0)
            w_load(0)
            x_load(1)
            w_load(1)
            x_quant(0)
            w_quant(0)
            x_load(2)
            w_load(2)
            x_quant(1)
            w_quant(1)
            x_load(3)
            w_load(3)
            x_quant(2)
            w_quant(2)
            x_quant(3)
            w_load(4)
            w_quant(3)
            w_quant(4)

            NACT = 5   # phase-1 active W block set
            xg_next = 4
            units_p1 = [(ob, ch) for ch in range(NTC) for ob in range(NACT)]
            for i, (ob, ch) in enumerate(units_p1):
                mm_unit(ob, ch)
                # two x groups after each early unit: all 12 remaining groups
                # are issued within the first 6 units of phase 1
                for _ in range(2):
                    if xg_next < NG:
                        x_load(xg_next)
                        x_quant(xg_next)
                        xg_next += 1
                if i == 12:
                    # stage the first phase-2 W load (lw slots are free; later
                    # loads must wait their predecessor's quant or they clog
                    # the scalar ring ahead of the output stores)
                    w_load(NACT)
            for ob in range(NACT, NB):
                w_quant(ob)
                if ob + 1 < NB:
                    w_load(ob + 1)
                for ch in range(NTC):
                    mm_unit(ob, ch)
    nc.compile()
    return nc


_NC_CACHE = {}
LAST_EXEC_NS = None


def _get_nc():
    if "full" not in _NC_CACHE:
        _NC_CACHE["full"] = build_nc()
    return _NC_CACHE["full"]


def _run(x, weight, bias, trace=False):
    global LAST_EXEC_NS
    x = np.asarray(x, dtype=np.float32).reshape(T_FULL, I)
    weight = np.asarray(weight, dtype=np.float32)
    bias = np.asarray(bias, dtype=np.float32)

    TT = T_FULL // TSPLIT
    TO = O // OSPLIT
    in_maps = []
    for c in range(N_CORES):
        ti, oj = divmod(c, OSPLIT)
        in_maps.append(
            {
                "x": np.ascontiguousarray(x[ti * TT : (ti + 1) * TT, :]),
                "w": np.ascontiguousarray(weight[oj * TO : (oj + 1) * TO, :]),
                "bias": np.ascontiguousarray(bias[oj * TO : (oj + 1) * TO]),
            }
        )

    nc = _get_nc()
    res = run_bass_kernel_spmd(
        nc, in_maps, core_ids=list(range(N_CORES)), trace=trace
    )
    LAST_EXEC_NS = res.exec_time_ns

    out = np.empty((T_FULL, O), dtype=np.float32)
    for c in range(N_CORES):
        ti, oj = divmod(c, OSPLIT)
        out[ti * TT : (ti + 1) * TT, oj * TO : (oj + 1) * TO] = np.asarray(
            res.results[c]["out"]
        ).astype(np.float32).T
    return out.reshape(B, S, O)


def kernel(x, weight, bias):
    return _run(x, weight, bias, trace=False)


def kernel_traced(x, weight, bias):
    _run(x, weight, bias, trace=True)
    return LAST_EXEC_NS


# revision 20
# speedup vs baseline: 1.9009x; 1.9009x over previous
# BitLinear (eval path) Trainium2 kernel: ternary weight quant + int8-grade
# activation quant + dense matmul, tensor-parallel over 8 NeuronCores.
#
# Math (per reference):
#   w_scale[o] = max(mean_k |W[o,k]|, EPS)
#   w_quant    = clip(round(W / w_scale), -1, 1)            (ternary, exact)
#   x_scale[t] = max(max_k |x[t,k]| / 127, EPS)
#   xq[t,k]    = bf16(x[t,k] / x_scale[t])   -- bf16 grid is finer than the
#                int8 grid below 128; skipping the integer round costs ~9e-3
#                rel err vs the int8 reference (budget 2e-2).
#   outT[o,t]  = (sum_k wq[o,k] * xq[t,k]) * ws[o] * xs[t] + bias[o]
#   (outputs computed/stored transposed [O,T] in bf16; host transposes+upcasts)
#
# W ternary stays exact: magic-constant rounding (v + 1.5*2^23), scalar act1
# on ScalarE, act2 (-M -> bf16) on GpSimd, ternary clip fused into the
# post-transpose fp8 cast on DVE.
#
# KEY STRUCTURE (learned from traces of 5 prior revisions):
# - The matmul runs W-STATIONARY / X-MOVING producing outT[o,t]. Each W block
#   (128 rows) then feeds 128 matmuls (~27us of PE work), so the W quant
#   pipeline (1 block per ~11-20us) always stays ahead of the PE -- with the
#   old X-stationary layout each W block fed only 6.8us and the frontend
#   paced the whole kernel (~780us for a 437us-roofline matmul).
# - ws becomes a per-partition scalar in the epilogue (no broadcast needed);
#   xs needs one [1,128]->[128,128] ones-matmul broadcast per token group.
# - All of XQ is resident (128KB/partition bf16, K-major); W streams.
# - Queue split: SWDGE(gpsimd)=x loads; scalar HWDGE=W loads+stores;
#   sync HWDGE=all xbar transposes; frontend at half-K granularity.
import numpy as np

import concourse.bacc as bacc
import concourse.bass as bass
import concourse.tile as tile
from concourse import mybir
from concourse.bass_utils import run_bass_kernel_spmd
from concourse.masks import make_identity

F32 = mybir.dt.float32
BF16 = mybir.dt.bfloat16
FP8 = mybir.dt.float8e4

EPS = 1e-5
MAGIC = 12582912.0  # 1.5 * 2^23

B, S, I, O = 4, 2048, 4096, 4096
T_FULL = B * S
TSPLIT, OSPLIT = 4, 2
N_CORES = TSPLIT * OSPLIT

A = mybir.AluOpType
AF = mybir.ActivationFunctionType


def build_nc(K=I, TO=O // OSPLIT, TT=T_FULL // TSPLIT):
    """Per-core program: x [TT, K], w [TO, K], bias [TO] -> outT [TO, TT]."""
    KT = K // 128      # 32 k subtiles
    KH = K // 2        # 2048: half-K frontend granularity
    KTH = KT // 2      # 16 k subtiles per half
    GT = 128           # tokens / out-rows per group
    NG = TT // GT      # 16 token groups
    NB = TO // GT      # 16 W blocks
    TC = 512           # moving width per matmul (tokens)
    NTC = TT // TC     # 4 token chunks

    nc = bacc.Bacc("TRN2", target_bir_lowering=False, debug=False)
    x_d = nc.dram_tensor("x", [TT, K], F32, kind="ExternalInput").ap()
    w_d = nc.dram_tensor("w", [TO, K], F32, kind="ExternalInput").ap()
    bias_d = nc.dram_tensor("bias", [TO], F32, kind="ExternalInput").ap()
    out_d = nc.dram_tensor("out", [TO, TT], BF16, kind="ExternalOutput").ap()

    with tile.TileContext(nc) as tc:
        with (
            tc.tile_pool(name="lx", bufs=2) as p_lx,      # f32 x half-row loads
            tc.tile_pool(name="lw", bufs=2) as p_lw,      # f32 W half-row loads
            tc.tile_pool(name="btx", bufs=2) as p_btx,    # bf16 quantized x halves
            tc.tile_pool(name="btw", bufs=1) as p_btw,    # bf16 quantized W halves
            tc.tile_pool(name="wst", bufs=1) as p_wst,    # transposed W staging
            tc.tile_pool(name="wq", bufs=5) as p_wq,      # streamed fp8 W blocks
            tc.tile_pool(name="xq", bufs=1) as p_xq,      # resident K-major xq
            tc.tile_pool(name="sml", bufs=6) as p_sml,
            tc.tile_pool(name="osb", bufs=3) as p_osb,
            tc.tile_pool(name="const", bufs=1) as p_const,
            tc.tile_pool(name="ps_mm", bufs=5, space="PSUM") as ps_mm,
            tc.tile_pool(name="ps_tr", bufs=1, space="PSUM") as ps_tr,
        ):
            ident = p_const.tile([128, 128], F32)
            make_identity(nc, ident[:])
            ones_row = p_const.tile([1, 128], BF16)
            nc.vector.memset(ones_row[:], 1.0)
            mag_col = p_const.tile([128, 1], F32)
            nc.vector.memset(mag_col[:], MAGIC)
            nmag_col = p_const.tile([128, 1], F32)
            nc.vector.memset(nmag_col[:], -MAGIC)
            xs_cols = p_const.tile([128, NG], F32)    # x_scale, t on partitions
            ws_cols = p_const.tile([128, NB], F32)    # w_scale, o on partitions
            xs_bc = p_const.tile([128, TT], BF16)     # x_scale bcast rows
            bias_cols = p_const.tile([128, NB], F32)  # bias, o on partitions
            nc.sync.dma_start(
                out=bias_cols[:],
                in_=bass.AP(
                    tensor=bias_d.tensor, offset=bias_d.offset,
                    ap=[[1, 128], [128, NB]],
                ),
            )

            xq_ch = [
                p_xq.tile([128, KT, TC], BF16, name=f"xq_{c}") for c in range(NTC)
            ]
            wq_blocks = {}
            lx_tiles = {}
            lw_tiles = {}

            # ---------- loads: half-K tiles, each flow on its own queue ------
            def x_load(tg):
                for h in range(2):
                    xg = p_lx.tile([128, KH], F32, tag="lx")
                    nc.gpsimd.dma_start(
                        out=xg[:],
                        in_=x_d[tg * GT : (tg + 1) * GT, h * KH : (h + 1) * KH],
                    )
                    lx_tiles[(tg, h)] = xg

            def w_load(ob):
                for h in range(2):
                    wg = p_lw.tile([128, KH], F32, tag="lw")
                    nc.scalar.dma_start(
                        out=wg[:],
                        in_=w_d[ob * GT : (ob + 1) * GT, h * KH : (h + 1) * KH],
                    )
                    lw_tiles[(ob, h)] = wg

            # ---------- x group: amax, one-pass quantize, transpose, bcast --
            def x_quant(tg):
                xga = lx_tiles.pop((tg, 0))
                xgb = lx_tiles.pop((tg, 1))
                ama = p_sml.tile([128, 1], F32, tag="am")
                amb = p_sml.tile([128, 1], F32, tag="am")
                nc.vector.tensor_reduce(
                    out=ama[:], in_=xga[:], axis=mybir.AxisListType.X,
                    op=A.max, apply_absolute_value=True,
                )
                nc.vector.tensor_reduce(
                    out=amb[:], in_=xgb[:], axis=mybir.AxisListType.X,
                    op=A.max, apply_absolute_value=True,
                )
                am = p_sml.tile([128, 1], F32, tag="am")
                nc.vector.tensor_tensor(out=am[:], in0=ama[:], in1=amb[:], op=A.max)
                nc.vector.tensor_scalar(
                    out=xs_cols[:, tg : tg + 1], in0=am[:],
                    scalar1=1.0 / 127.0, scalar2=EPS, op0=A.mult, op1=A.max,
                )
                rxs = p_sml.tile([128, 1], F32, tag="rxs")
                nc.vector.reciprocal(rxs[:], xs_cols[:, tg : tg + 1])
                # xs column -> [1,128] row -> K=1 ones-matmul broadcast -> xs_bc
                ptr = ps_tr.tile([1, 128], F32, tag="tr")
                nc.tensor.transpose(ptr[:], xs_cols[:, tg : tg + 1], ident[:])
                xsrow = p_sml.tile([1, 128], BF16, tag="xsrow")
                nc.vector.tensor_copy(xsrow[:], ptr[:])
                pbc = ps_tr.tile([128, 128], F32, tag="bc")
                nc.tensor.matmul(pbc[:], ones_row[:], xsrow[:], start=True, stop=True)
                nc.vector.tensor_copy(xs_bc[:, tg * GT : (tg + 1) * GT], pbc[:])
                # xq = bf16(x * (1/xs)) in ONE scalar pass, then xbar transpose
                ch, tsl = tg // (TC // GT), (tg % (TC // GT)) * GT
                for h, xg in ((0, xga), (1, xgb)):
                    xot = p_btx.tile([128, KH], BF16, tag="btx")
                    nc.scalar.activation(
                        out=xot[:], in_=xg[:], func=AF.Identity, scale=rxs[:],
                    )
                    nc.sync.dma_start_transpose(
                        xq_ch[ch][:, h * KTH : (h + 1) * KTH, tsl : tsl + GT],
                        xot[:],
                    )

            # ---------- W block: scales, ternary, transpose, clip+fp8 -------
            def w_quant(ob):
                wga = lw_tiles.pop((ob, 0))
                wgb = lw_tiles.pop((ob, 1))
                wsa = p_sml.tile([128, 1], F32, tag="ws")
                wsb = p_sml.tile([128, 1], F32, tag="ws")
                nc.vector.tensor_reduce(
                    out=wsa[:], in_=wga[:], axis=mybir.AxisListType.X,
                    op=A.add, apply_absolute_value=True,
                )
                nc.vector.tensor_reduce(
                    out=wsb[:], in_=wgb[:], axis=mybir.AxisListType.X,
                    op=A.add, apply_absolute_value=True,
                )
                wsum = p_sml.tile([128, 1], F32, tag="ws")
                nc.vector.tensor_tensor(out=wsum[:], in0=wsa[:], in1=wsb[:], op=A.add)
                nc.vector.tensor_scalar(
                    out=ws_cols[:, ob : ob + 1], in0=wsum[:],
                    scalar1=1.0 / K, scalar2=EPS, op0=A.mult, op1=A.max,
                )
                rws = p_sml.tile([128, 1], F32, tag="ws")
                nc.vector.reciprocal(rws[:], ws_cols[:, ob : ob + 1])
                # u = w*(1/ws) + M (exact rint, ScalarE); u - M -> bf16 on
                # GpSimd; ternary clip fused into the fp8 cast on DVE
                wqb = p_wq.tile([128, KT, GT], FP8, tag="wq")
                for h, wg in ((0, wga), (1, wgb)):
                    nc.scalar.activation(
                        out=wg[:], in_=wg[:], func=AF.Identity,
                        scale=rws[:], bias=mag_col[:],
                    )
                    wot = p_btw.tile([128, KH], BF16, tag="btw")
                    nc.scalar.activation(
                        out=wot[:], in_=wg[:], func=AF.Identity, bias=nmag_col[:],
                    )
                    wstg = p_wst.tile([128, KTH, GT], BF16, tag="wst")
                    nc.sync.dma_start_transpose(wstg[:], wot[:])
                    nc.vector.tensor_scalar(
                        out=wqb[:, h * KTH : (h + 1) * KTH, :], in0=wstg[:],
                        scalar1=1.0, scalar2=-1.0, op0=A.min, op1=A.max,
                    )
                wq_blocks[ob] = wqb

            # ---------- matmul unit: one (W block, token chunk) ----------
            def mm_unit(ob, ch):
                wqb = wq_blocks[ob]
                pm = ps_mm.tile([128, TC], F32, tag="mm")
                for kt in range(KT):
                    nc.tensor.matmul(
                        pm[:],
                        wqb[:, kt, :],
                        xq_ch[ch][:, kt, :],
                        start=(kt == 0),
                        stop=(kt == KT - 1),
                    )
                osb = p_osb.tile([128, TC], BF16, tag="osb")
                nc.vector.scalar_tensor_tensor(
                    out=osb[:], in0=pm[:], scalar=ws_cols[:, ob : ob + 1],
                    in1=xs_bc[:, ch * TC : (ch + 1) * TC], op0=A.mult, op1=A.mult,
                )
                nc.scalar.activation(
                    out=osb[:], in_=osb[:], func=AF.Identity,
                    bias=bias_cols[:, ob : ob + 1],
                )
                nc.scalar.dma_start(
                    out=out_d[ob * GT : (ob + 1) * GT, ch * TC : (ch + 1) * TC],
                    in_=osb[:],
                )

            # ---------- main schedule ----------
            # Phase 1: W blocks 0-4 revisited ch-major — the PE sweeps the
            # active 5-block set once per x chunk, arriving at chunk ch just
            # after its x groups land. Block liveness stays within wq bufs=5.
            # Phase 2: blocks 5-15 block-major — each W block covers 27us of
            # PE work vs ~12-17us production, so W stays ahead.
            # X groups are front-loaded hard: they gate the chunk sweeps.
            x_load(0)
            w_load(0)
            x_load(1)
            w_load(1)
            x_quant(0)
            w_quant(0)
            x_load(2)
            w_load(2)
            x_quant(1)
            w_quant(1)
            x_load(3)
            w_load(3)
            x_quant(2)
            w_quant(2)
            x_quant(3)
            w_load(4)
            w_quant(3)
            w_quant(4)

            NACT = 5   # phase-1 active W block set
            xg_next = 4
            units_p1 = [(ob, ch) for ch in range(NTC) for ob in range(NACT)]
            for i, (ob, ch) in enumerate(units_p1):
                mm_unit(ob, ch)
                # two x groups after each early unit: all 12 remaining groups
                # are issued within the first 6 units of phase 1
                for _ in range(2):
                    if xg_next < NG:
                        x_load(xg_next)
                        x_quant(xg_next)
                        xg_next += 1
                if i == 12:
                    # stage the first phase-2 W load (lw slots are free; later
                    # loads must wait their predecessor's quant or they clog
                    # the scalar ring ahead of the output stores)
                    w_load(NACT)
            for ob in range(NACT, NB):
                w_quant(ob)
                if ob + 1 < NB:
                    w_load(ob + 1)
                for ch in range(NTC):
                    mm_unit(ob, ch)
    nc.compile()
    return nc


_NC_CACHE = {}
LAST_EXEC_NS = None


def _get_nc():
    if "full" not in _NC_CACHE:
        _NC_CACHE["full"] = build_nc()
    return _NC_CACHE["full"]


def _run(x, weight, bias, trace=False):
    global LAST_EXEC_NS
    x = np.asarray(x, dtype=np.float32).reshape(T_FULL, I)
    weight = np.asarray(weight, dtype=np.float32)
    bias = np.asarray(bias, dtype=np.float32)

    TT = T_FULL // TSPLIT
    TO = O // OSPLIT
    in_maps = []
    for c in range(N_CORES):
        ti, oj = divmod(c, OSPLIT)
        in_maps.append(
            {
                "x": np.ascontiguousarray(x[ti * TT : (ti + 1) * TT, :]),
                "w": np.ascontiguousarray(weight[oj * TO : (oj + 1) * TO, :]),
                "bias": np.ascontiguousarray(bias[oj * TO : (oj + 1) * TO]),
            }
        )

    nc = _get_nc()
    res = run_bass_kernel_spmd(
        nc, in_maps, core_ids=list(range(N_CORES)), trace=trace
    )
    LAST_EXEC_NS = res.exec_time_ns

    out = np.empty((T_FULL, O), dtype=np.float32)
    for c in range(N_CORES):
        ti, oj = divmod(c, OSPLIT)
        out[ti * TT : (ti + 1) * TT, oj * TO : (oj + 1) * TO] = np.asarray(
            res.results[c]["out"]
        ).astype(np.float32).T
    return out.reshape(B, S, O)


def kernel(x, weight, bias):
    return _run(x, weight, bias, trace=False)


def kernel_traced(x, weight, bias):
    _run(x, weight, bias, trace=True)
    return LAST_EXEC_NS


# revision 22
# speedup vs baseline: 2.0433x; 1.0749x over previous
# BitLinear (eval path) Trainium2 kernel: ternary weight quant + int8-grade
# activation quant + dense matmul, tensor-parallel over 8 NeuronCores.
#
# Math (per reference):
#   w_scale[o] = max(mean_k |W[o,k]|, EPS)
#   w_quant    = clip(round(W / w_scale), -1, 1)            (ternary, exact)
#   x_scale[t] = max(max_k |x[t,k]| / 127, EPS)
#   xq[t,k]    = bf16(x[t,k] / x_scale[t])   -- bf16 grid is finer than the
#                int8 grid below 128; ~9e-3 rel err vs int8 ref (budget 2e-2)
#   out[t,o]   = (sum_k xq[t,k] * wq[o,k]) * xs[t] * ws[o] + bias[o]
#   (outputs stored bf16, host upcasts: +~2e-3 in quadrature)
#
# Layout: out[t,o] with xq stationary ([128k,128t] tiles) and fp8 ternary W
# moving 512-wide; W fully SBUF-resident (64KB/partition fp8), xq tiles
# stream through a 5-slot pool. Frontend at half-K granularity.
#
# THE key scheduling fix (found via traces of 7 prior revisions): all
# frontend chains (loads, reduces, quant activations, xbar transposes,
# fp8 casts) are emitted under tc.high_priority(), so the Tile scheduler
# never queues them behind the mm stream's epilogue/store ops on shared
# engines. Without this, late-emitted frontend stages inherit late
# program-order priority and each chain hop eats 3-15us of queue wait --
# the frontend then paces the whole kernel (~780us for a 437us-roofline
# matmul).
#
# Queue split: SWDGE(gpsimd) = x loads; scalar HWDGE ring = W loads +
# output stores; sync HWDGE ring = all xbar transposes. Epilogue:
# DVE scalar_tensor_tensor (psum*xs*ws_epi) + gpsimd bias add.
import numpy as np

import concourse.bacc as bacc
import concourse.bass as bass
import concourse.tile as tile
from concourse import mybir
from concourse.bass_utils import run_bass_kernel_spmd
from concourse.masks import make_identity

F32 = mybir.dt.float32
BF16 = mybir.dt.bfloat16
FP8 = mybir.dt.float8e4

EPS = 1e-5
MAGIC = 12582912.0  # 1.5 * 2^23

B, S, I, O = 4, 2048, 4096, 4096
T_FULL = B * S
TSPLIT, OSPLIT = 4, 2
N_CORES = TSPLIT * OSPLIT

A = mybir.AluOpType
AF = mybir.ActivationFunctionType
PRIO = 1 << 20  # frontend priority offset


def build_nc(K=I, TO=O // OSPLIT, TT=T_FULL // TSPLIT):
    """Per-core program: x [TT, K], w [TO, K], bias [TO] -> out [TT, TO]."""
    KT = K // 128      # 32 k subtiles
    KH = K // 2        # half-K frontend granularity
    KTH = KT // 2      # 16 k subtiles per half
    GT = 128           # tokens / out-rows per group
    NG = TT // GT      # 16 token groups
    NB = TO // GT      # 16 W blocks
    OC = 512           # moving width per matmul
    NOC = TO // OC     # 4 o-chunks

    nc = bacc.Bacc("TRN2", target_bir_lowering=False, debug=False)
    x_d = nc.dram_tensor("x", [TT, K], F32, kind="ExternalInput").ap()
    w_d = nc.dram_tensor("w", [TO, K], F32, kind="ExternalInput").ap()
    bias_d = nc.dram_tensor("bias", [TO], F32, kind="ExternalInput").ap()
    out_d = nc.dram_tensor("out", [TT, TO], BF16, kind="ExternalOutput").ap()

    with tile.TileContext(nc) as tc:
        with (
            tc.tile_pool(name="lx", bufs=4) as p_lx,      # f32 x half-row loads
            tc.tile_pool(name="lw", bufs=5) as p_lw,      # f32 W half-row loads
            tc.tile_pool(name="btx", bufs=2) as p_btx,    # bf16 quantized x halves
            tc.tile_pool(name="btw", bufs=1) as p_btw,    # bf16 quantized W halves
            tc.tile_pool(name="wst", bufs=1) as p_wst,    # transposed W staging
            tc.tile_pool(name="wq", bufs=1) as p_wq,      # resident fp8 weights
            tc.tile_pool(name="xq", bufs=5) as p_xq,      # bf16 K-major token tiles
            tc.tile_pool(name="sml", bufs=6) as p_sml,
            tc.tile_pool(name="osb", bufs=3) as p_osb,
            tc.tile_pool(name="const", bufs=1) as p_const,
            tc.tile_pool(name="ps_mm", bufs=5, space="PSUM") as ps_mm,
            tc.tile_pool(name="ps_tr", bufs=1, space="PSUM") as ps_tr,
        ):
            ident = p_const.tile([128, 128], F32)
            make_identity(nc, ident[:])
            ones_row = p_const.tile([1, 128], BF16)
            nc.vector.memset(ones_row[:], 1.0)
            mag_col = p_const.tile([128, 1], F32)
            nc.vector.memset(mag_col[:], MAGIC)
            nmag_col = p_const.tile([128, 1], F32)
            nc.vector.memset(nmag_col[:], -MAGIC)
            xs_cols = p_const.tile([128, NG], F32)    # x_scale, t on partitions
            ws_epi = p_const.tile([128, TO], BF16)    # w_scale bcast rows
            bias_bc = p_const.tile([128, TO], BF16)   # bias bcast rows
            nc.gpsimd.dma_start(
                out=bias_bc[:],
                in_=bass.AP(
                    tensor=bias_d.tensor, offset=bias_d.offset,
                    ap=[[0, 128], [1, TO]],
                ),
            )

            wq_oc = [
                p_wq.tile([128, KT, OC], FP8, name=f"wq_{oc}") for oc in range(NOC)
            ]
            xq_tiles = {}
            lx_tiles = {}
            lw_tiles = {}

            # ---------- frontend (all emitted at high priority) ----------
            def x_load(tg):
                with tc.high_priority(offset=PRIO):
                    for h in range(2):
                        xg = p_lx.tile([128, KH], F32, tag="lx")
                        nc.gpsimd.dma_start(
                            out=xg[:],
                            in_=x_d[tg * GT : (tg + 1) * GT, h * KH : (h + 1) * KH],
                        )
                        lx_tiles[(tg, h)] = xg

            def w_load(ob):
                with tc.high_priority(offset=PRIO):
                    for h in range(2):
                        wg = p_lw.tile([128, KH], F32, tag="lw")
                        nc.scalar.dma_start(
                            out=wg[:],
                            in_=w_d[ob * GT : (ob + 1) * GT, h * KH : (h + 1) * KH],
                        )
                        lw_tiles[(ob, h)] = wg

            def x_quant(tg):
                with tc.high_priority(offset=PRIO):
                    xga = lx_tiles.pop((tg, 0))
                    xgb = lx_tiles.pop((tg, 1))
                    ama = p_sml.tile([128, 1], F32, tag="am")
                    amb = p_sml.tile([128, 1], F32, tag="am")
                    nc.vector.tensor_reduce(
                        out=ama[:], in_=xga[:], axis=mybir.AxisListType.X,
                        op=A.max, apply_absolute_value=True,
                    )
                    nc.vector.tensor_reduce(
                        out=amb[:], in_=xgb[:], axis=mybir.AxisListType.X,
                        op=A.max, apply_absolute_value=True,
                    )
                    am = p_sml.tile([128, 1], F32, tag="am")
                    nc.vector.tensor_tensor(
                        out=am[:], in0=ama[:], in1=amb[:], op=A.max
                    )
                    nc.vector.tensor_scalar(
                        out=xs_cols[:, tg : tg + 1], in0=am[:],
                        scalar1=1.0 / 127.0, scalar2=EPS, op0=A.mult, op1=A.max,
                    )
                    rxs = p_sml.tile([128, 1], F32, tag="rxs")
                    nc.vector.reciprocal(rxs[:], xs_cols[:, tg : tg + 1])
                    xq_t = p_xq.tile([128, KT, GT], BF16, tag="xq")
                    for h, xg in ((0, xga), (1, xgb)):
                        xot = p_btx.tile([128, KH], BF16, tag="btx")
                        nc.scalar.activation(
                            out=xot[:], in_=xg[:], func=AF.Identity, scale=rxs[:],
                        )
                        nc.sync.dma_start_transpose(
                            xq_t[:, h * KTH : (h + 1) * KTH, :], xot[:]
                        )
                    xq_tiles[tg] = xq_t

            def w_quant(ob):
                with tc.high_priority(offset=PRIO):
                    wga = lw_tiles.pop((ob, 0))
                    wgb = lw_tiles.pop((ob, 1))
                    wsa = p_sml.tile([128, 1], F32, tag="ws")
                    wsb = p_sml.tile([128, 1], F32, tag="ws")
                    nc.vector.tensor_reduce(
                        out=wsa[:], in_=wga[:], axis=mybir.AxisListType.X,
                        op=A.add, apply_absolute_value=True,
                    )
                    nc.vector.tensor_reduce(
                        out=wsb[:], in_=wgb[:], axis=mybir.AxisListType.X,
                        op=A.add, apply_absolute_value=True,
                    )
                    wsum = p_sml.tile([128, 1], F32, tag="ws")
                    nc.vector.tensor_tensor(
                        out=wsum[:], in0=wsa[:], in1=wsb[:], op=A.add
                    )
                    wsf = p_sml.tile([128, 1], F32, tag="ws")
                    nc.vector.tensor_scalar(
                        out=wsf[:], in0=wsum[:], scalar1=1.0 / K, scalar2=EPS,
                        op0=A.mult, op1=A.max,
                    )
                    rws = p_sml.tile([128, 1], F32, tag="ws")
                    nc.vector.reciprocal(rws[:], wsf[:])
                    # ws column -> [1,128] row -> K=1 ones-matmul broadcast
                    ptr = ps_tr.tile([1, 128], F32, tag="tr")
                    nc.tensor.transpose(ptr[:], wsf[:], ident[:])
                    wsrow = p_sml.tile([1, 128], BF16, tag="wsrow")
                    nc.vector.tensor_copy(wsrow[:], ptr[:])
                    pbc = ps_tr.tile([128, 128], F32, tag="bc")
                    nc.tensor.matmul(
                        pbc[:], ones_row[:], wsrow[:], start=True, stop=True
                    )
                    nc.vector.tensor_copy(ws_epi[:, ob * GT : (ob + 1) * GT], pbc[:])
                    # u = w*(1/ws) + M (exact rint); u - M -> bf16; ternary
                    # clip fused into the fp8 cast after the transpose
                    oc, osl = ob // (OC // GT), (ob % (OC // GT)) * GT
                    for h, wg in ((0, wga), (1, wgb)):
                        nc.scalar.activation(
                            out=wg[:], in_=wg[:], func=AF.Identity,
                            scale=rws[:], bias=mag_col[:],
                        )
                        wot = p_btw.tile([128, KH], BF16, tag="btw")
                        nc.scalar.activation(
                            out=wot[:], in_=wg[:], func=AF.Identity,
                            bias=nmag_col[:],
                        )
                        wstg = p_wst.tile([128, KTH, GT], BF16, tag="wst")
                        nc.sync.dma_start_transpose(wstg[:], wot[:])
                        nc.vector.tensor_scalar(
                            out=wq_oc[oc][:, h * KTH : (h + 1) * KTH,
                                          osl : osl + GT],
                            in0=wstg[:],
                            scalar1=1.0, scalar2=-1.0, op0=A.min, op1=A.max,
                        )

            # ---------- matmul pass (natural priority) ----------
            def mm_pass(tg, oc):
                xq_t = xq_tiles[tg]
                pm = ps_mm.tile([128, OC], F32, tag="mm")
                for kt in range(KT):
                    nc.tensor.matmul(
                        pm[:],
                        xq_t[:, kt, :],
                        wq_oc[oc][:, kt, :],
                        start=(kt == 0),
                        stop=(kt == KT - 1),
                    )
                osb = p_osb.tile([128, OC], BF16, tag="osb")
                nc.vector.scalar_tensor_tensor(
                    out=osb[:], in0=pm[:], scalar=xs_cols[:, tg : tg + 1],
                    in1=ws_epi[:, oc * OC : (oc + 1) * OC], op0=A.mult, op1=A.mult,
                )
                nc.gpsimd.tensor_tensor(
                    out=osb[:], in0=osb[:],
                    in1=bias_bc[:, oc * OC : (oc + 1) * OC], op=A.add,
                )
                nc.scalar.dma_start(
                    out=out_d[tg * GT : (tg + 1) * GT, oc * OC : (oc + 1) * OC],
                    in_=osb[:],
                )

            # ---------- main schedule ----------
            x_load(0)
            w_load(0)
            x_load(1)
            w_load(1)
            x_quant(0)
            w_quant(0)
            x_load(2)
            w_load(2)
            x_quant(1)
            w_quant(1)
            x_load(3)
            w_load(3)
            x_quant(2)
            w_quant(2)
            x_quant(3)
            w_load(4)
            w_load(5)
            w_quant(3)
            # Section 0, oc-major; W blocks 4-15 stream through the mm shadow.
            wb_next = 4
            wl_next = 6
            for oc in range(NOC):
                for tg in range(4):
                    mm_pass(tg, oc)
                    if wb_next < NB:
                        w_quant(wb_next)
                        wb_next += 1
                    if wl_next < NB:
                        w_load(wl_next)
                        wl_next += 1
            # Section 0 oc3 freed xq 0-3 -> prefetch; sections 1-3 tg-major.
            for g in (4, 5, 6, 7):
                x_load(g)
                x_quant(g)
            for sec in range(1, 4):
                for tg in range(sec * 4, sec * 4 + 4):
                    for oc in range(NOC):
                        mm_pass(tg, oc)
                    nxt = tg + 4
                    if nxt < NG:
                        x_load(nxt)
                        x_quant(nxt)
    nc.compile()
    return nc


_NC_CACHE = {}
LAST_EXEC_NS = None


def _get_nc():
    if "full" not in _NC_CACHE:
        _NC_CACHE["full"] = build_nc()
    return _NC_CACHE["full"]


def _run(x, weight, bias, trace=False):
    global LAST_EXEC_NS
    x = np.asarray(x, dtype=np.float32).reshape(T_FULL, I)
    weight = np.asarray(weight, dtype=np.float32)
    bias = np.asarray(bias, dtype=np.float32)

    TT = T_FULL // TSPLIT
    TO = O // OSPLIT
    in_maps = []
    for c in range(N_CORES):
        ti, oj = divmod(c, OSPLIT)
        in_maps.append(
            {
                "x": np.ascontiguousarray(x[ti * TT : (ti + 1) * TT, :]),
                "w": np.ascontiguousarray(weight[oj * TO : (oj + 1) * TO, :]),
                "bias": np.ascontiguousarray(bias[oj * TO : (oj + 1) * TO]),
            }
        )

    nc = _get_nc()
    res = run_bass_kernel_spmd(
        nc, in_maps, core_ids=list(range(N_CORES)), trace=trace
    )
    LAST_EXEC_NS = res.exec_time_ns

    out = np.empty((T_FULL, O), dtype=np.float32)
    for c in range(N_CORES):
        ti, oj = divmod(c, OSPLIT)
        out[ti * TT : (ti + 1) * TT, oj * TO : (oj + 1) * TO] = np.asarray(
            res.results[c]["out"]
        ).astype(np.float32)
    return out.reshape(B, S, O)


def kernel(x, weight, bias):
    return _run(x, weight, bias, trace=False)


def kernel_traced(x, weight, bias):
    _run(x, weight, bias, trace=True)
    return LAST_EXEC_NS
